# revision 1
# baseline (speedup 1.0000x reference)
"""Trainium2 Bass kernel for nn_DensityModulatedAttention (B=2, L=2048, D=768, H=12).

Sharding (8 NeuronCores): core i -> batch b=i//4, global heads {3*(i%4)+k}
for k in 0..2 (head parallel attention), query quarter i%4 for the output
projection.  One SPMD NEFF: QKV projection + RMSNorm + RoPE + attention run
head-local; three AllToAll collectives (one per local head, replica groups
[0..3] and [4..7]) re-shard attention output from head-split to query-split;
the output projection then runs fully local.

Math notes (validated against the jax reference in numpy_proto.py):
  - density bias is a per-query additive constant broadcast over keys, so it
    cancels in softmax; it is intentionally not applied.
  - softmax is computed without max-subtraction (scores are O(5) after
    RMSNorm, exp cannot overflow).
  - q_scale/k_scale are folded into the RoPE coefficient tables on the host;
    1/sqrt(hd) and q's RMS scale are folded into q; k's RMS scale is folded
    into the per-partition `scale` operand of the softmax exp() activation.
  - attention probabilities and V are bf16 (matmul-rate identical, halves
    SBUF); everything else fp32 with fp32r (FP22) matmuls.
"""
import os
import ml_dtypes
import numpy as np
from contextlib import ExitStack

import concourse.bass as bass
import concourse.tile as tile
from concourse import bacc, mybir
from concourse.bass_utils import run_bass_kernel_spmd

dt = mybir.dt
F32 = dt.float32
F32R = dt.float32r
BF16 = dt.bfloat16

B, L, D, H, HD = 2, 2048, 768, 12, 64
NC = 8
HL = 3            # local heads per core
QTR = 512         # query quarter owned for projection
NLT = L // 128    # 16 l-tiles
CC = D // 128     # 6 contraction chunks
SCALE = HD ** -0.5
REPLICA_GROUPS = [[0, 1, 2, 3, 4, 5, 6, 7]]


def _bc(ap2d, n):
    """Insert a zero-stride broadcast dim of size n between partition and free."""
    return bass.AP(ap2d.tensor, ap2d.offset, [list(ap2d.ap[0]), [0, n], list(ap2d.ap[-1])])


def _view3(ap2d, step, n, inner, extra_off=0):
    """(128, X) slice -> (128, n, inner) with free dims [(step, n), (1, inner)]."""
    return bass.AP(ap2d.tensor, ap2d.offset + extra_off,
                   [list(ap2d.ap[0]), [step, n], [1, inner]])


def kernel_body(ctx: ExitStack, tc: tile.TileContext, outs, ins):
    nc = tc.nc
    out_d = outs['out']
    xT_d, wqkvT_d = ins['xT'], ins['wqkvT']
    pw_d, projb_d, ident_d = ins['pw_rounds'], ins['projb'], ins['ident']

    MUL = mybir.AluOpType.mult
    ADD = mybir.AluOpType.add
    Sqrt = mybir.ActivationFunctionType.Sqrt
    Square = mybir.ActivationFunctionType.Square
    Exp = mybir.ActivationFunctionType.Exp

    const = ctx.enter_context(tc.tile_pool(name="const", bufs=1))
    stat = ctx.enter_context(tc.tile_pool(name="stat", bufs=1))
    kv = ctx.enter_context(tc.tile_pool(name="kv", bufs=1))
    dram = ctx.enter_context(tc.tile_pool(name="dram", bufs=1, space="DRAM"))
    scr = ctx.enter_context(tc.tile_pool(name="scr", bufs=3))
    ktp = ctx.enter_context(tc.tile_pool(name="ktp", bufs=1))

    ident_sb = const.tile([128, 128], F32, tag="ident")
    nc.sync.dma_start(ident_sb[:], ident_d[:])
    projb_sb = const.tile([128, D], F32, tag="projb")
    nc.sync.dma_start(projb_sb[:], projb_d[:])
    rows = const.tile([1, L], F32, tag="rows")
    inv_row = const.tile([1, L], F32, tag="inv")

    # stats col layout: t*6 + h for q, t*6 + 3 + h for k  (group-contiguous)
    ms = stat.tile([128, 96], F32, tag="ms")
    rr = stat.tile([128, 96], F32, tag="rr")
    nrt = stat.tile([128, 96], F32, tag="nrt")

    # roped q in (l, d) layout; vts: bf16 [v_h | 1.0] per head; kT per head
    qro, vts = [], []
    for t in range(NLT):
        qro.append(kv.tile([128, HL * HD], F32, tag=f"qro{t}", name=f"qro{t}"))
        vts.append(kv.tile([128, HL * 65], BF16, tag=f"vts{t}", name=f"vts{t}"))
    # kT2: per head (128, 1024); cols 128u..+128 = key-tile pair (2u, 2u+1),
    # partition rows 0:64 = kT of even tile, 64:128 = kT of odd tile
    kT = [ktp.tile([128, L // 2], F32R, tag=f"kT{h}", name=f"kT{h}") for h in range(HL)]

    # ---------------- phase 1: QKV projection + rope + k transposes ----------
    with tc.tile_pool(name="xw", bufs=1) as xw, \
         tc.tile_pool(name="qkv_ps", bufs=2, space="PSUM") as qkv_ps:
        xts, ws = [], []
        for i in range(CC):
            xt = xw.tile([128, L], F32R, tag=f"xt{i}", name=f"xt{i}")
            nc.sync.dma_start(xt[:], xT_d[128 * i:128 * (i + 1), :].bitcast(F32R))
            xts.append(xt)
            w = xw.tile([128, 576], F32R, tag=f"w{i}", name=f"w{i}")
            nc.sync.dma_start(w[:], wqkvT_d[128 * i:128 * (i + 1), :].bitcast(F32R))
            ws.append(w)
        pe_sb = {}
        for name, dten in (('aq', ins['pe_aq']), ('bq', ins['pe_bq']),
                           ('ak', ins['pe_ak']), ('bk', ins['pe_bk'])):
            t = xw.tile([128, NLT * HD], F32, tag=f"pe{name}", name=f"pe{name}")
            nc.sync.dma_start(_view3(t, HD, NLT, HD), dten.rearrange("(t p) d -> p t d", p=128))
            pe_sb[name] = t
        kro = [xw.tile([128, HL * HD], F32, tag=f"kro{t}", name=f"kro{t}")
               for t in range(NLT)]

        for g in range(4):
            for s4 in range(4):
                t = 4 * g + s4
                # columns: A = [k(192) | v(0:96)], B = [v(96:192) | q(192)]
                psA = qkv_ps.tile([128, 288], F32, tag="qkvA", bufs=3)
                psB = qkv_ps.tile([128, 288], F32, tag="qkvB", bufs=3)
                for c in range(CC):
                    lhsT = xts[c][:, 128 * t:128 * (t + 1)]
                    nc.tensor.matmul(psA[:], lhsT, ws[c][:, 0:288],
                                     start=(c == 0), stop=(c == CC - 1))
                    nc.tensor.matmul(psB[:], lhsT, ws[c][:, 288:576],
                                     start=(c == 0), stop=(c == CC - 1))
                kslice = psA[:, 0:192]
                qslice = psB[:, 96:288]
                # stats: sum(x^2) per (l, head); square on ACT (idle in phase 1)
                sqscr = scr.tile([128, 384], F32, tag="sq")
                nc.scalar.activation(sqscr[:, 0:192], kslice, Square)
                nc.scalar.activation(sqscr[:, 192:384], qslice, Square)
                nc.vector.tensor_reduce(
                    bass.AP(ms.tensor, ms.offset + 6 * t + 3, [list(ms.ap[0]), [1, HL]]),
                    _view3(sqscr, 64, HL, 64), axis=mybir.AxisListType.X, op=ADD)
                nc.vector.tensor_reduce(
                    bass.AP(ms.tensor, ms.offset + 6 * t, [list(ms.ap[0]), [1, HL]]),
                    _view3(sqscr[:, 192:384], 64, HL, 64), axis=mybir.AxisListType.X, op=ADD)
                # k rope: kro = pe_ak*k + pe_bk*swap(k)  (rrms_k folded into exp)
                ka = scr.tile([128, 192], F32, tag="ka")
                pk = pe_sb['ak'][:, HD * t:HD * (t + 1)]
                nc.vector.tensor_mul(_view3(ka, 64, HL, 64), _view3(kslice, 64, HL, 64), _bc(pk, HL))
                kb = scr.tile([128, 192], F32, tag="kb")
                pb = pe_sb['bk'][:, HD * t:HD * (t + 1)]
                for s in (0, 1):
                    nc.vector.tensor_mul(
                        bass.AP(kb.tensor, kb.offset + s, [list(kb.ap[0]), [64, HL], [2, 32]]),
                        bass.AP(kslice.tensor, kslice.offset + (1 - s), [list(kslice.ap[0]), [64, HL], [2, 32]]),
                        bass.AP(pb.tensor, pb.offset + s, [list(pb.ap[0]), [0, HL], [2, 32]]))
                nc.gpsimd.tensor_add(kro[t][:], ka[:], kb[:])
                # q raw evac into qro (roped in place once this group's stats land)
                nc.vector.tensor_copy(qro[t][:], qslice)
                # v evac -> bf16 augmented [v_h | 1.0]; v is split 96/96 over A/B
                nc.vector.tensor_copy(vts[t][:, 0:64], psA[:, 192:256])
                nc.vector.tensor_copy(vts[t][:, 65:97], psA[:, 256:288])
                nc.vector.tensor_copy(vts[t][:, 97:129], psB[:, 0:32])
                nc.vector.tensor_copy(vts[t][:, 130:194], psB[:, 32:96])
                nc.vector.memset(bass.AP(vts[t].tensor, vts[t].offset + 64,
                                         [list(vts[t].ap[0]), [65, HL], [1, 1]]), 1.0)
            # ---- per-group stats finalize: rr = c/sqrt(ms), Newton-refined --
            cg = slice(24 * g, 24 * g + 24)
            nc.vector.tensor_scalar(out=ms[:, cg], in0=ms[:, cg], scalar1=1.0 / HD,
                                    scalar2=1e-6, op0=MUL, op1=ADD)
            nc.vector.reciprocal(nrt[:, cg], ms[:, cg])
            nc.scalar.activation(rr[:, cg], nrt[:, cg], Sqrt)
            nc.vector.tensor_mul(nrt[:, cg], rr[:, cg], rr[:, cg])
            nc.vector.tensor_mul(nrt[:, cg], nrt[:, cg], ms[:, cg])
            nc.vector.tensor_scalar(out=nrt[:, cg], in0=nrt[:, cg], scalar1=-1.0,
                                    scalar2=3.0, op0=MUL, op1=ADD)
            nc.vector.tensor_mul(rr[:, cg], rr[:, cg], nrt[:, cg])
            for off, cconst in ((0, 0.5 * SCALE), (3, 0.5)):  # q cols, k cols
                nc.vector.tensor_scalar(
                    out=bass.AP(rr.tensor, rr.offset + 24 * g + off,
                                [list(rr.ap[0]), [6, 4], [1, 3]]),
                    in0=bass.AP(rr.tensor, rr.offset + 24 * g + off,
                                [list(rr.ap[0]), [6, 4], [1, 3]]),
                    scalar1=cconst, scalar2=None, op0=MUL)
            # ---- q rope in place for this group ----
            for t in range(4 * g, 4 * g + 4):
                qn = scr.tile([128, 192], F32, tag="qn")
                rrq = bass.AP(rr.tensor, rr.offset + 6 * t, [list(rr.ap[0]), [1, HL], [0, 64]])
                nc.vector.tensor_mul(_view3(qn, 64, HL, 64), _view3(qro[t], 64, HL, 64), rrq)
                qa = scr.tile([128, 192], F32, tag="ka")
                pq = pe_sb['aq'][:, HD * t:HD * (t + 1)]
                nc.vector.tensor_mul(_view3(qa, 64, HL, 64), _view3(qn, 64, HL, 64), _bc(pq, HL))
                qb = scr.tile([128, 192], F32, tag="kb")
                pqb = pe_sb['bq'][:, HD * t:HD * (t + 1)]
                for s in (0, 1):
                    nc.vector.tensor_mul(
                        bass.AP(qb.tensor, qb.offset + s, [list(qb.ap[0]), [64, HL], [2, 32]]),
                        bass.AP(qn.tensor, qn.offset + (1 - s), [list(qn.ap[0]), [64, HL], [2, 32]]),
                        bass.AP(pqb.tensor, pqb.offset + s, [list(pqb.ap[0]), [0, HL], [2, 32]]))
                nc.gpsimd.tensor_add(qro[t][:], qa[:], qb[:])
            # ---- k transposes for this group (all heads), PE, into kT2 ----
            # pair layout: even tiles -> rows 0:64 (direct DVE evac), odd
            # tiles -> rows 64:128 via an SBUF bounce + DMA (transpose-mode
            # matmuls must output at PSUM partition 0, and DVE cannot cross
            # partitions -- only DMA can)
            for h in range(HL):
                tpk = qkv_ps.tile([64, 512], F32, tag="tpk")
                for i_e, s4 in enumerate((0, 2, 1, 3)):
                    t = 4 * g + s4
                    nc.tensor.transpose(tpk[:, 128 * i_e:128 * (i_e + 1)],
                                        kro[t][:, 64 * h:64 * (h + 1)], ident_sb[:])
                nc.vector.tensor_copy(kT[h][0:64, 256 * g:256 * (g + 1)], tpk[:, 0:256])
                kscr = scr.tile([64, 256], F32R, tag="kscr", bufs=2)
                nc.vector.tensor_copy(kscr[:], tpk[:, 256:512])
                nc.sync.dma_start(kT[h][64:128, 256 * g:256 * (g + 1)], kscr[:])

    # ---------------- phase 2: attention + A2A + projection ------------------
    qtp = ctx.enter_context(tc.tile_pool(name="qtp", bufs=2))
    att2 = ctx.enter_context(tc.tile_pool(name="att2", bufs=1))
    expp = ctx.enter_context(tc.tile_pool(name="expp", bufs=6))
    out_sb = [att2.tile([128, D], F32, tag=f"osb{lt}", name=f"osb{lt}") for lt in range(4)]

    # PSUM: "sc" slots (2 x 2 banks) shared by scores halves / transposes /
    # projection; "av" slots (4 x 1 bank) hold the per-chunk AV accumulators.
    sc_ps = ctx.enter_context(tc.tile_pool(name="sc_ps", bufs=2, space="PSUM"))
    av_ps = ctx.enter_context(tc.tile_pool(name="av_ps", bufs=4, space="PSUM"))

    def proj_round(h):
        # projection round h: c-chunk c covers recv blocks (2c, 2c+1) of the
        # h-th A2A; wrong-batch blocks have zero weights (host-supplied)
        outbuf = a2a_out[h]
        prjs, pws = [], []
        for c in range(4):
            prj = scr.tile([128, QTR], BF16, tag=f"prj{c}", bufs=1)
            nc.sync.dma_start(prj[0:64, :], outbuf[2 * c])
            nc.sync.dma_start(prj[64:128, :], outbuf[2 * c + 1])
            prjs.append(prj)
            pw = scr.tile([128, D], BF16, tag=f"pw{c}", bufs=1)
            nc.sync.dma_start(pw[:], pw_d[h, c])
            pws.append(pw)
        for lt in range(4):
            for e in range(2):
                pp = sc_ps.tile([128, 384], F32, tag="sc")
                for c in range(4):
                    nc.tensor.matmul(pp[:], prjs[c][:, 128 * lt:128 * (lt + 1)],
                                     pws[c][:, 384 * e:384 * (e + 1)],
                                     start=(c == 0), stop=(c == 3))
                dst = out_sb[lt][:, 384 * e:384 * (e + 1)]
                src1 = projb_sb[:, 384 * e:384 * (e + 1)] if h == 0 else dst
                nc.vector.tensor_add(dst, pp[:], src1)

    a2a_out = []
    for h in range(HL):
        # q transposes for this head (PE, sc slots)
        qTt = qtp.tile([128, L], F32R, tag="qT", name=f"qT{h}")
        for u in range(2):
            tpq = sc_ps.tile([64, 1024], F32, tag="sc")
            for s in range(8):
                t = 8 * u + s
                nc.tensor.transpose(tpq[:, 128 * s:128 * (s + 1)],
                                    qro[t][:, 64 * h:64 * (h + 1)], ident_sb[:])
            nc.vector.tensor_copy(qTt[0:64, 1024 * u:1024 * (u + 1)], tpq[:])
        nc.sync.dma_start(qTt[64:128, :], qTt[0:64, :])
        # scores/exp in half-tiles, AV interleaved per m-tile
        avs = [av_ps.tile([65, 512], F32, tag="av", name=f"av{h}_{c2}") for c2 in range(4)]
        for u in range(NLT // 2):
            exs = [expp.tile([128, L], BF16, tag="exp", name=f"ex{p}") for p in range(2)]
            for half in range(2):
                schs = []
                for p in range(2):  # p: even/odd key-tile of the pair
                    sch = sc_ps.tile([128, 1024], F32, tag="sc")
                    for c in (0, 1):
                        nc.tensor.matmul(
                            sch[:, 512 * c:512 * (c + 1)],
                            kT[h][64 * p:64 * p + 64, 128 * u:128 * (u + 1)],
                            qTt[64 * p:64 * p + 64,
                                1024 * half + 512 * c:1024 * half + 512 * (c + 1)],
                            start=True, stop=True, tile_position=(64 * p, 0))
                    schs.append(sch)
                for p in range(2):
                    j = 2 * u + p
                    nc.scalar.activation(exs[p][:, 1024 * half:1024 * (half + 1)], schs[p][:],
                                         Exp, scale=rr[:, 6 * j + 3 + h:6 * j + 3 + h + 1])
                    for c in (0, 1):
                        c2 = 2 * half + c
                        nc.tensor.matmul(avs[c2][:], vts[j][:, 65 * h:65 * h + 65],
                                         exs[p][:, 512 * c2:512 * (c2 + 1)],
                                         start=(j == 0), stop=(j == NLT - 1))
        # normalization: rows 0..63 unnormalized attnT, row 64 rowsum
        attnT = att2.tile([64, L], BF16, tag="attnT")
        for c2 in range(4):
            av = avs[c2]
            nc.vector.tensor_copy(rows[:, 512 * c2:512 * (c2 + 1)], av[64:65, :])
            rstat = scr.tile([128, 4], F32, tag="rstat")
            nc.sync.dma_start(rstat[:], bass.AP(rows.tensor, rows.offset + 512 * c2,
                                                [list(rows.ap[0]), [4, 128], [1, 4]]))
            nc.vector.reciprocal(rstat[:], rstat[:])
            nc.sync.dma_start(bass.AP(inv_row.tensor, inv_row.offset + 512 * c2,
                                      [list(inv_row.ap[0]), [4, 128], [1, 4]]), rstat[:])
            bcr = scr.tile([64, 512], F32, tag="bcr", bufs=2)
            nc.gpsimd.partition_broadcast(bcr[:], inv_row[0:1, 512 * c2:512 * (c2 + 1)])
            nc.vector.tensor_mul(attnT[:, 512 * c2:512 * (c2 + 1)], av[0:64, :], bcr[:])
        # A2A across all 8 ranks: quarter (j%4) mirrored to both batch groups
        inbuf = dram.tile([8, 64, QTR], BF16, tag=f"a2ai{h}", name=f"a2ai{h}")
        outbuf = dram.tile([8, 64, QTR], BF16, tag=f"a2ao{h}", name=f"a2ao{h}")
        for j in range(8):
            nc.sync.dma_start(inbuf[j], attnT[:, QTR * (j % 4):QTR * (j % 4 + 1)])
        nc.gpsimd.collective_compute(
            "AllToAll", mybir.AluOpType.bypass, replica_groups=REPLICA_GROUPS,
            ins=[inbuf.opt()], outs=[outbuf.opt()])
        a2a_out.append(outbuf)
        # defer projection round h-1 to here: its A2A had a full attention
        # phase to complete, so the in-order PE queue never waits on it
        if h >= 1:
            proj_round(h - 1)
    proj_round(HL - 1)

    for lt in range(4):
        nc.sync.dma_start(out_d[128 * lt:128 * (lt + 1), :], out_sb[lt][:])


# ============================ host side ======================================

def host_prep(x, density_weights, pe, qkv_w, q_scale, k_scale, proj_w, proj_b,
              density_scale, density_bias):
    x = np.ascontiguousarray(np.asarray(x, dtype=np.float32))
    pe = np.asarray(pe, dtype=np.float32)
    qkv_w = np.asarray(qkv_w, dtype=np.float32)
    q_scale = np.asarray(q_scale, dtype=np.float32)
    k_scale = np.asarray(k_scale, dtype=np.float32)
    proj_w = np.asarray(proj_w, dtype=np.float32)
    proj_b = np.asarray(proj_b, dtype=np.float32)

    pe_ = pe[0, 0]
    pe_a = np.empty((L, HD), np.float32)
    pe_b = np.empty((L, HD), np.float32)
    pe_a[:, 0::2] = pe_[:, :, 0, 0]
    pe_a[:, 1::2] = pe_[:, :, 1, 1]
    pe_b[:, 0::2] = pe_[:, :, 0, 1]
    pe_b[:, 1::2] = pe_[:, :, 1, 0]
    swap = np.arange(HD) ^ 1
    pe_aq = np.ascontiguousarray(pe_a * q_scale[None, :])
    pe_bq = np.ascontiguousarray(pe_b * q_scale[swap][None, :])
    pe_ak = np.ascontiguousarray(pe_a * k_scale[None, :])
    pe_bk = np.ascontiguousarray(pe_b * k_scale[swap][None, :])

    Wq, Wk, Wv = qkv_w[0:D], qkv_w[D:2 * D], qkv_w[2 * D:3 * D]
    ident = np.eye(128, dtype=np.float32)
    projb = np.ascontiguousarray(np.broadcast_to(proj_b[None, :], (128, D))).astype(np.float32)

    in_maps = []
    for core in range(NC):
        b, j = core // 4, core % 4
        heads = [3 * j + k for k in range(HL)]
        xT = np.ascontiguousarray(x[b].T)
        # wqkvT columns: [k(192) | v(0:96)] then [v(96:192) | q(192)]
        kcols = [Wk[hh * HD:(hh + 1) * HD, :].T for hh in heads]
        qcols = [Wq[hh * HD:(hh + 1) * HD, :].T for hh in heads]
        vfull = np.concatenate([Wv[hh * HD:(hh + 1) * HD, :].T for hh in heads], axis=1)
        wqkvT = np.ascontiguousarray(np.concatenate(
            kcols + [vfull[:, 0:96], vfull[:, 96:192]] + qcols, axis=1))
        pw = np.zeros((HL, 4, 128, D), np.float32)
        for k in range(HL):
            for c in range(4):
                for half, jabs in ((0, 2 * c), (1, 2 * c + 1)):
                    if jabs // 4 != b:
                        continue  # wrong-batch block: weights stay zero
                    hh = 3 * (jabs % 4) + k
                    pw[k, c, 64 * half:64 * (half + 1)] = proj_w[:, hh * HD:(hh + 1) * HD].T
        in_maps.append({
            'xT': xT, 'wqkvT': wqkvT,
            'pe_aq': pe_aq, 'pe_bq': pe_bq, 'pe_ak': pe_ak, 'pe_bk': pe_bk,
            'pw_rounds': np.ascontiguousarray(pw).astype(ml_dtypes.bfloat16), 'projb': projb, 'ident': ident,
        })
    return in_maps


_PROGRAM = None


def build_program():
    global _PROGRAM
    if _PROGRAM is not None:
        return _PROGRAM
    nc = bacc.Bacc("TRN2", target_bir_lowering=False, debug=False, num_devices=NC)
    ins = {
        'xT': nc.dram_tensor("xT", [D, L], F32, kind="ExternalInput").ap(),
        'wqkvT': nc.dram_tensor("wqkvT", [D, 576], F32, kind="ExternalInput").ap(),
        'pe_aq': nc.dram_tensor("pe_aq", [L, HD], F32, kind="ExternalInput").ap(),
        'pe_bq': nc.dram_tensor("pe_bq", [L, HD], F32, kind="ExternalInput").ap(),
        'pe_ak': nc.dram_tensor("pe_ak", [L, HD], F32, kind="ExternalInput").ap(),
        'pe_bk': nc.dram_tensor("pe_bk", [L, HD], F32, kind="ExternalInput").ap(),
        'pw_rounds': nc.dram_tensor("pw_rounds", [HL, 4, 128, D], BF16, kind="ExternalInput").ap(),
        'projb': nc.dram_tensor("projb", [128, D], F32, kind="ExternalInput").ap(),
        'ident': nc.dram_tensor("ident", [128, 128], F32, kind="ExternalInput").ap(),
    }
    outs = {'out': nc.dram_tensor("out", [QTR, D], F32, kind="ExternalOutput").ap()}
    with tile.TileContext(nc) as tc:
        with ExitStack() as ctx:
            kernel_body(ctx, tc, outs, ins)
    nc.compile()
    _PROGRAM = nc
    return nc


def kernel(**inputs) -> np.ndarray:
    nc = build_program()
    in_maps = host_prep(**inputs)
    res = run_bass_kernel_spmd(nc, in_maps, core_ids=list(range(NC)),
                               trace=bool(int(os.environ.get("KERNEL_TRACE", "0"))))
    out = np.empty((B, L, D), np.float32)
    for core in range(NC):
        b, j = core // 4, core % 4
        out[b, QTR * j:QTR * (j + 1), :] = res.results[core]['out']
    kernel.last_results = res
    return out



# revision 17
# speedup vs baseline: 1.7978x; 1.7978x over previous
"""Trainium2 Bass kernel for nn_DensityModulatedAttention (B=2, L=2048, D=768, H=12).

Sharding (8 NeuronCores): core i -> batch b=i//4, global heads {3*(i%4)+k}
for k in 0..2 (head parallel attention), query quarter i%4 for the output
projection.  One SPMD NEFF: QKV projection + RMSNorm + RoPE + attention run
head-local; three AllToAll collectives (one per local head) re-shard the
attention output from head-split to query-split; the output projection then
runs fully local.

Perf structure (v3):
  - fp16 everywhere except PSUM accumulation, stats and the final output
    (fp16's 11-bit mantissa keeps softmax logits accurate; bf16 fails the
    2e-2 gate).
  - q/k head dims are de-interleaved to [re(32)|im(32)] on the host (weight
    row permutation) so RoPE is a contiguous half-swap on DVE.
  - all transposes are batched XBAR DMA transposes (one per l-tile, 6 head
    blocks each); no PE transposes, no PSUM bounce.
  - scores matmuls are full-array 128-contraction: the transposed k/q tiles
    are zero-padded in rows 64:128, which keeps the PE activity monitor fed
    (measured: 64-row matmuls never reach the 2.4 GHz clock state).
  - scores (f32 PSUM) are cast to fp16 SBUF on DVE, one (128,2048) exp per
    key tile on ACT; AV accumulates in fp32 PSUM with an appended
    ones-column producing the softmax row-sums.
  - DMA issues cost ~630ns of queue time each; bulk transfers are batched
    into single multi-block access patterns and spread across the gpsimd /
    sync / scalar queues.
  - softmax normalization for head h is emitted in two chunks inside head
    h+1's loop (reciprocal round-trip first, broadcast+muls 3 tiles later)
    so its DVE ops never head-of-line block the attention casts; AV matmuls
    run 6 tiles behind scores for the same reason.
  - density bias is a per-query additive constant -> cancels in softmax.
"""
import os
import ml_dtypes
import numpy as np
from contextlib import ExitStack

import concourse.bass as bass
import concourse.tile as tile
from concourse import bacc, mybir
from concourse.bass_utils import run_bass_kernel_spmd

dt = mybir.dt
F32 = dt.float32
F16 = dt.float16

B, L, D, H, HD = 2, 2048, 768, 12, 64
NC = 8
HL = 3            # local heads per core
QTR = 512         # query quarter owned for projection
NLT = L // 128    # 16 l-tiles
CC = D // 128     # 6 contraction chunks
SCALE = HD ** -0.5
REPLICA_GROUPS = [[0, 1, 2, 3, 4, 5, 6, 7]]
AVLAG = 6         # AV matmuls trail scores by this many key tiles


def _bc(ap2d, n):
    """Insert a zero-stride broadcast dim of size n between partition and free."""
    return bass.AP(ap2d.tensor, ap2d.offset, [list(ap2d.ap[0]), [0, n], list(ap2d.ap[-1])])


def _view3(ap2d, step, n, inner, extra_off=0):
    """(128, X) slice -> (128, n, inner) with free dims [(step, n), (1, inner)]."""
    return bass.AP(ap2d.tensor, ap2d.offset + extra_off,
                   [list(ap2d.ap[0]), [step, n], [1, inner]])


def _swapv(ap2d, step, n, extra_off=0):
    """(128, X) slice -> per-'step'-block half-swapped view: cols [32:64] then
    [0:32] of each block (free dims [(step,n),(-32,2),(1,32)] at offset+32)."""
    return bass.AP(ap2d.tensor, ap2d.offset + extra_off + 32,
                   [list(ap2d.ap[0]), [step, n], [-32, 2], [1, 32]])


def kernel_body(ctx: ExitStack, tc: tile.TileContext, outs, ins):
    nc = tc.nc
    out_d = outs['out']
    xT_d, wqkvT_d = ins['xT'], ins['wqkvT']
    pw_d, projb_d = ins['pw_rounds'], ins['projb']

    MUL = mybir.AluOpType.mult
    ADD = mybir.AluOpType.add
    Sqrt = mybir.ActivationFunctionType.Sqrt
    Square = mybir.ActivationFunctionType.Square
    Exp = mybir.ActivationFunctionType.Exp

    const = ctx.enter_context(tc.tile_pool(name="const", bufs=1))
    stat = ctx.enter_context(tc.tile_pool(name="stat", bufs=1))
    kv = ctx.enter_context(tc.tile_pool(name="kv", bufs=1))
    dram = ctx.enter_context(tc.tile_pool(name="dram", bufs=1, space="DRAM"))
    scr = ctx.enter_context(tc.tile_pool(name="scr", bufs=3))

    projb_sb = const.tile([128, D], F32, tag="projb")
    nc.gpsimd.dma_start(projb_sb[:], projb_d[:])
    rows = const.tile([1, L], F32, tag="rows")
    inv_row = const.tile([1, L], F32, tag="inv")
    expbias = const.tile([128, 1], F32, tag="expbias")
    nc.vector.memset(expbias[:], -9.0)

    # stats col layout: t*6 + h for q, t*6 + 3 + h for k  (group-contiguous)
    ms = stat.tile([128, 96], F32, tag="ms")
    rr = stat.tile([128, 96], F32, tag="rr")
    nrt = stat.tile([128, 96], F32, tag="nrt")

    # persistent attention operands
    # vts[t]: (128, 3*128) fp16, head block h = [v_h(64) | 1.0 | 0*63]
    # kqro[t]: (128, 6*128) fp16 rope output, block b: b=h -> [k_h(64)|0*64],
    #          b=3+h -> [q_h(64)|0*64]
    # kqT: (128, 6*2048) fp16, block b col 2048*b+128*t = XBAR transpose of
    #          kqro[t] block b (rows 64:128 zero)
    vts, kqro, qro = [], [], []
    for t in range(NLT):
        vts.append(kv.tile([128, HL * 128], F16, tag=f"vts{t}", name=f"vts{t}"))
        kqro.append(kv.tile([128, 6 * 128], F16, tag=f"kqro{t}", name=f"kqro{t}"))
        qro.append(kv.tile([128, HL * HD], F16, tag=f"qro{t}", name=f"qro{t}"))
    kqT = kv.tile([128, 6 * L], F16, tag="kqT", name="kqT")

    # zero the pad columns once (transposed zeros become the zero pad rows)
    for t in range(NLT):
        nc.vector.memset(_view3(kqro[t], 128, 6, 64, extra_off=64), 0.0)
        nc.vector.memset(_view3(vts[t], 128, HL, 63, extra_off=65), 0.0)
        nc.vector.memset(_view3(vts[t], 128, HL, 1, extra_off=64), 1.0)

    # ---------------- phase 1: QKV projection + rope + transposes ------------
    with tc.tile_pool(name="xw", bufs=1) as xw, \
         tc.tile_pool(name="qkv_ps", bufs=1, space="PSUM") as qkv_ps:
        # input DMAs, x ordered by l-column group so matmuls can start early
        xts = [xw.tile([128, L], F16, tag=f"xt{i}", name=f"xt{i}") for i in range(CC)]
        for i in range(CC):
            nc.scalar.dma_start(xts[i][:, 0:1024], xT_d[128 * i:128 * (i + 1), 0:1024])
        ws = []
        for i in range(CC):
            w = xw.tile([128, 576], F16, tag=f"w{i}", name=f"w{i}")
            nc.sync.dma_start(w[:], wqkvT_d[128 * i:128 * (i + 1), :])
            ws.append(w)
        for i in range(CC):
            nc.gpsimd.dma_start(xts[i][:, 1024:2048], xT_d[128 * i:128 * (i + 1), 1024:2048])
        pe_sb = {}
        for name in ('dq', 'cq', 'dk', 'ck'):
            tl = xw.tile([128, NLT * HD], F16, tag=f"pe{name}", name=f"pe{name}")
            nc.gpsimd.dma_start(tl[:], ins['pe_' + name][:])
            pe_sb[name] = tl

        # column layout: psA = [k(192) | v_h0(64)], psB = [v_h1 | v_h2 | q(192)]
        for g in range(4):
            for s4 in range(4):
                t = 4 * g + s4
                psA = qkv_ps.tile([128, 512], F32, tag="qkvA", bufs=4)
                psB = qkv_ps.tile([128, 512], F32, tag="qkvB", bufs=4)
                for c in range(CC):
                    lhsT = xts[c][:, 128 * t:128 * (t + 1)]
                    nc.tensor.matmul(psA[:, 0:256], lhsT, ws[c][:, 0:256],
                                     start=(c == 0), stop=(c == CC - 1))
                    nc.tensor.matmul(psB[:, 0:320], lhsT, ws[c][:, 256:576],
                                     start=(c == 0), stop=(c == CC - 1))
                kslice = psA[:, 0:192]
                qslice = psB[:, 128:320]
                # stats: sum(x^2) per (l, head); square on ACT, reduce on DVE
                sqscr = scr.tile([128, 384], F16, tag="sq")
                nc.scalar.activation(sqscr[:, 0:192], kslice, Square)
                nc.scalar.activation(sqscr[:, 192:384], qslice, Square)
                nc.vector.tensor_reduce(
                    bass.AP(ms.tensor, ms.offset + 6 * t + 3, [list(ms.ap[0]), [1, HL]]),
                    _view3(sqscr, 64, HL, 64), axis=mybir.AxisListType.X, op=ADD)
                nc.vector.tensor_reduce(
                    bass.AP(ms.tensor, ms.offset + 6 * t, [list(ms.ap[0]), [1, HL]]),
                    _view3(sqscr[:, 192:384], 64, HL, 64), axis=mybir.AxisListType.X, op=ADD)
                # k rope directly from PSUM (rrms_k folded into exp scale):
                # kro_h = diag_k * k + cross_k * halfswap(k)
                ka = scr.tile([128, 192], F16, tag="ka")
                pk = pe_sb['dk'][:, HD * t:HD * (t + 1)]
                nc.vector.tensor_mul(_view3(ka, 64, HL, 64), _view3(kslice, 64, HL, 64), _bc(pk, HL))
                kb = scr.tile([128, 192], F16, tag="kb")
                pck = pe_sb['ck'][:, HD * t:HD * (t + 1)]
                nc.vector.tensor_mul(_view3(kb, 64, HL, 64), _swapv(kslice, 64, HL), _bc(pck, HL))
                nc.gpsimd.tensor_add(_view3(kqro[t], 128, HL, 64), _view3(ka, 64, HL, 64),
                                     _view3(kb, 64, HL, 64))
                # q raw evac on ACT (roped after this group's stats land)
                nc.scalar.activation(qro[t][:], qslice, mybir.ActivationFunctionType.Copy)
                # v evac -> vts head blocks
                nc.vector.tensor_copy(
                    bass.AP(vts[t].tensor, vts[t].offset, [list(vts[t].ap[0]), [1, 64]]),
                    psA[:, 192:256])
                nc.vector.tensor_copy(_view3(vts[t], 128, 2, 64, extra_off=128),
                                      _view3(psB, 64, 2, 64))
            # ---- per-group stats finalize: rr = 2/sqrt(ms), Newton-refined --
            cg = slice(24 * g, 24 * g + 24)
            nc.vector.tensor_scalar(out=ms[:, cg], in0=ms[:, cg], scalar1=1.0 / HD,
                                    scalar2=1e-6, op0=MUL, op1=ADD)
            nc.vector.reciprocal(nrt[:, cg], ms[:, cg])
            nc.scalar.activation(rr[:, cg], nrt[:, cg], Sqrt)
            nc.vector.tensor_mul(nrt[:, cg], rr[:, cg], rr[:, cg])
            nc.vector.tensor_mul(nrt[:, cg], nrt[:, cg], ms[:, cg])
            nc.vector.tensor_scalar(out=nrt[:, cg], in0=nrt[:, cg], scalar1=-1.0,
                                    scalar2=3.0, op0=MUL, op1=ADD)
            nc.vector.tensor_mul(rr[:, cg], rr[:, cg], nrt[:, cg])
            for off, cconst in ((0, 0.5 * SCALE), (3, 0.5)):  # q cols, k cols
                nc.vector.tensor_scalar(
                    out=bass.AP(rr.tensor, rr.offset + 24 * g + off,
                                [list(rr.ap[0]), [6, 4], [1, 3]]),
                    in0=bass.AP(rr.tensor, rr.offset + 24 * g + off,
                                [list(rr.ap[0]), [6, 4], [1, 3]]),
                    scalar1=cconst, scalar2=None, op0=MUL)
            # ---- q rope + batched XBAR transpose for this group ----
            for t in range(4 * g, 4 * g + 4):
                qn = scr.tile([128, 192], F16, tag="qn")
                rrq = bass.AP(rr.tensor, rr.offset + 6 * t, [list(rr.ap[0]), [1, HL], [0, 64]])
                nc.vector.tensor_mul(_view3(qn, 64, HL, 64), _view3(qro[t], 64, HL, 64), rrq)
                qa = scr.tile([128, 192], F16, tag="ka")
                pq = pe_sb['dq'][:, HD * t:HD * (t + 1)]
                nc.vector.tensor_mul(_view3(qa, 64, HL, 64), _view3(qn, 64, HL, 64), _bc(pq, HL))
                qb = scr.tile([128, 192], F16, tag="kb")
                pcq = pe_sb['cq'][:, HD * t:HD * (t + 1)]
                nc.vector.tensor_mul(_view3(qb, 64, HL, 64), _swapv(qn, 64, HL), _bc(pcq, HL))
                nc.vector.tensor_add(_view3(kqro[t], 128, HL, 64, extra_off=384),
                                     _view3(qa, 64, HL, 64), _view3(qb, 64, HL, 64))
                nc.sync.dma_start(
                    bass.AP(kqT.tensor, kqT.offset + 128 * t,
                            [list(kqT.ap[0]), [L, 6], [1, 128]]),
                    kqro[t][:], transpose=True)

    # ---------------- phase 2: attention + A2A + projection ------------------
    att2 = ctx.enter_context(tc.tile_pool(name="att2", bufs=1))
    scbp = ctx.enter_context(tc.tile_pool(name="scbp", bufs=3))
    expp = ctx.enter_context(tc.tile_pool(name="expp", bufs=AVLAG + 2))
    out_sb = [att2.tile([128, D], F32, tag=f"osb{lt}", name=f"osb{lt}") for lt in range(4)]

    # PSUM: "sc" 2 slots x 2 banks (f32 score halves; proj rounds borrow),
    # "av" 4 slots x 1 bank (fp32 AV accumulators, row 64 = softmax row-sums).
    sc_ps = ctx.enter_context(tc.tile_pool(name="sc_ps", bufs=2, space="PSUM"))
    av_ps = ctx.enter_context(tc.tile_pool(name="av_ps", bufs=4, space="PSUM"))

    all_avs, attnTs, a2a_in, a2a_out = [], [], [], []
    for h in range(HL):
        inbuf = dram.tile([8, 64, QTR], F16, tag=f"a2ai{h}", name=f"a2ai{h}")
        outbuf = dram.tile([8, 64, QTR], F16, tag=f"a2ao{h}", name=f"a2ao{h}")
        a2a_in.append(inbuf)
        a2a_out.append(outbuf)
        attnTs.append(att2.tile([64, L], F16, tag=f"attnT{h}", name=f"attnT{h}"))

    def norm_pre(h):
        # row-sums -> (128,16) via DMA -> reciprocal -> back; nothing here
        # waits on a slow producer, so the DVE queue keeps flowing
        avs = all_avs[h]
        for c in range(4):
            nc.vector.tensor_copy(rows[:, 512 * c:512 * (c + 1)], avs[c][64:65, :])
        rstat = scr.tile([128, 16], F32, tag="rstat")
        nc.gpsimd.dma_start(rstat[:], bass.AP(rows.tensor, rows.offset,
                                              [list(rows.ap[0]), [16, 128], [1, 16]]))
        nc.vector.reciprocal(rstat[:], rstat[:])
        nc.gpsimd.dma_start(bass.AP(inv_row.tensor, inv_row.offset,
                                    [list(inv_row.ap[0]), [16, 128], [1, 16]]), rstat[:])
        bcr = scr.tile([64, L], F32, tag="bcr", bufs=1)
        nc.gpsimd.partition_broadcast(bcr[:], inv_row[0:1, :])
        return bcr

    def norm_post(h, bcr):
        # normalize (bcr is ready by now), ship to DRAM, trigger the A2A
        avs, attnT, inbuf = all_avs[h], attnTs[h], a2a_in[h]
        for c in range(4):
            nc.vector.tensor_mul(attnT[:, 512 * c:512 * (c + 1)], avs[c][0:64, :],
                                 bcr[:, 512 * c:512 * (c + 1)])
        for jg in range(8):
            nc.gpsimd.dma_start(inbuf[jg], attnT[:, QTR * (jg % 4):QTR * (jg % 4 + 1)])
        nc.gpsimd.collective_compute(
            "AllToAll", mybir.AluOpType.bypass, replica_groups=REPLICA_GROUPS,
            ins=[a2a_in[h].opt()], outs=[a2a_out[h].opt()])

    def proj_round(h):
        # projection round h: chunk c covers recv blocks (2c, 2c+1) of the
        # h-th A2A; wrong-batch blocks have zero weights (host-supplied)
        ob = a2a_out[h].opt()
        # per chunk: blocks (2c, 2c+1) are 128 contiguous rows of 512
        prjall = scr.tile([128, 4 * QTR], F16, tag="prjall", bufs=2)
        pwall = scr.tile([128, 4 * D], F16, tag="pwall", bufs=2)
        for c in range(4):
            nc.gpsimd.dma_start(
                prjall[:, QTR * c:QTR * (c + 1)],
                bass.AP(ob.tensor, ob.offset + c * 2 * 64 * QTR, [[QTR, 128], [1, QTR]]))
            nc.gpsimd.dma_start(
                pwall[:, D * c:D * (c + 1)],
                bass.AP(pw_d.tensor, pw_d.offset + (h * 4 + c) * 128 * D,
                        [[D, 128], [1, D]]))
        for lt in range(4):
            for e in range(2):
                pp = sc_ps.tile([128, 384], F32, tag="sc")
                for c in range(4):
                    nc.tensor.matmul(pp[:], prjall[:, 512 * c + 128 * lt:512 * c + 128 * (lt + 1)],
                                     pwall[:, 768 * c + 384 * e:768 * c + 384 * (e + 1)],
                                     start=(c == 0), stop=(c == 3))
                dst = out_sb[lt][:, 384 * e:384 * (e + 1)]
                src1 = projb_sb[:, 384 * e:384 * (e + 1)] if h == 0 else dst
                nc.vector.tensor_add(dst, pp[:], src1)
                if h == HL - 1 and e == 1:
                    nc.sync.dma_start(out_d[128 * lt:128 * (lt + 1), :], out_sb[lt][:])

    bcr_pend = None
    for h in range(HL):
        avs = [av_ps.tile([128, QTR], F32, tag="av", name=f"av{h}_{c}") for c in range(4)]
        all_avs.append(avs)
        pend = [None] * (AVLAG + 1)

        def flush_av(hh, jj, exv):
            for c in range(4):
                nc.tensor.matmul(avs[c][:], vts[jj][:, 128 * hh:128 * (hh + 1)],
                                 exv[:, 512 * c:512 * (c + 1)],
                                 start=(jj == 0), stop=(jj == NLT - 1))

        for j in range(NLT):
            if h >= 1 and j == 1:
                bcr_pend = norm_pre(h - 1)
            if h >= 1 and j == 4:
                norm_post(h - 1, bcr_pend)
            if h >= 2 and j == 7:
                proj_round(h - 2)
            scb = scbp.tile([128, L], F16, tag="scb")
            for half in range(2):
                sc = sc_ps.tile([128, 1024], F32, tag="sc")
                for c in range(2):
                    nc.tensor.matmul(
                        sc[:, 512 * c:512 * (c + 1)],
                        kqT[:, 2048 * h + 128 * j:2048 * h + 128 * (j + 1)],
                        kqT[:, 2048 * (HL + h) + 1024 * half + 512 * c:
                            2048 * (HL + h) + 1024 * half + 512 * (c + 1)],
                        start=True, stop=True)
                nc.vector.tensor_copy(scb[:, 1024 * half:1024 * (half + 1)], sc[:])
            # bias -9 guards fp16 overflow (scores reach ~18; exp caps at e^11);
            # the uniform e^-9 factor cancels in the softmax normalization
            ex = expp.tile([128, L], F16, tag="ex")
            nc.scalar.activation(ex[:], scb[:], Exp, bias=expbias[:],
                                 scale=rr[:, 6 * j + 3 + h:6 * j + 3 + h + 1])
            pend[j % (AVLAG + 1)] = (j, ex)
            if j >= AVLAG:
                flush_av(h, *pend[(j - AVLAG) % (AVLAG + 1)])
        for j in range(NLT - AVLAG, NLT):
            flush_av(h, *pend[j % (AVLAG + 1)])
    bcr_pend = norm_pre(HL - 1)
    norm_post(HL - 1, bcr_pend)
    proj_round(HL - 2)
    proj_round(HL - 1)


# ============================ host side ======================================

def host_prep(x, density_weights, pe, qkv_w, q_scale, k_scale, proj_w, proj_b,
              density_scale, density_bias):
    x = np.asarray(x, dtype=np.float32)
    pe = np.asarray(pe, dtype=np.float32)
    qkv_w = np.asarray(qkv_w, dtype=np.float32)
    q_scale = np.asarray(q_scale, dtype=np.float32)
    k_scale = np.asarray(k_scale, dtype=np.float32)
    proj_w = np.asarray(proj_w, dtype=np.float32)
    proj_b = np.asarray(proj_b, dtype=np.float32)

    # split-half de-interleave: new dim i<32 <- old 2i (even), 32+i <- old 2i+1
    perm = np.concatenate([np.arange(0, HD, 2), np.arange(1, HD, 2)])
    hswap = np.concatenate([np.arange(32, 64), np.arange(0, 32)])

    pe_ = pe[0, 0]  # (L, 32, 2, 2)
    diag = np.concatenate([pe_[:, :, 0, 0], pe_[:, :, 1, 1]], axis=1)  # (L, 64)
    cross = np.concatenate([pe_[:, :, 0, 1], pe_[:, :, 1, 0]], axis=1)
    qs = q_scale[perm]
    ks = k_scale[perm]

    def dev_pe(tbl):
        # (L, 64) -> on-chip (128, NLT*64): dev[p, 64*t + d] = tbl[128*t + p, d]
        return np.ascontiguousarray(
            tbl.reshape(NLT, 128, HD).transpose(1, 0, 2).reshape(128, NLT * HD)
        ).astype(np.float16)

    pe_dq = dev_pe(diag * qs[None, :])
    pe_cq = dev_pe(cross * qs[hswap][None, :])
    pe_dk = dev_pe(diag * ks[None, :])
    pe_ck = dev_pe(cross * ks[hswap][None, :])

    Wq, Wk, Wv = qkv_w[0:D], qkv_w[D:2 * D], qkv_w[2 * D:3 * D]
    projb = np.ascontiguousarray(np.broadcast_to(proj_b[None, :], (128, D))).astype(np.float32)

    in_maps = []
    for core in range(NC):
        b, jq = core // 4, core % 4
        heads = [3 * jq + k for k in range(HL)]
        xT = np.ascontiguousarray(x[b].T).astype(np.float16)
        # wqkvT columns: [k0p|k1p|k2p (192) | v0 (64)] then [v1|v2|q0p|q1p|q2p]
        kcols = [Wk[hh * HD:(hh + 1) * HD, :][perm].T for hh in heads]
        qcols = [Wq[hh * HD:(hh + 1) * HD, :][perm].T for hh in heads]
        vcols = [Wv[hh * HD:(hh + 1) * HD, :].T for hh in heads]
        wqkvT = np.ascontiguousarray(np.concatenate(
            kcols + [vcols[0], vcols[1], vcols[2]] + qcols, axis=1)).astype(np.float16)
        # proj: A2A h gives block s = head 3*(s%4)+h of rank s; chunk c has
        # blocks (2c, 2c+1) stacked on partitions; wrong-batch blocks get
        # zero weights
        pw = np.zeros((HL, 4, 128, D), np.float32)
        for k in range(HL):
            for c in range(4):
                for half, s in ((0, 2 * c), (1, 2 * c + 1)):
                    if s // 4 != b:
                        continue
                    hh = 3 * (s % 4) + k
                    pw[k, c, 64 * half:64 * (half + 1)] = proj_w[:, hh * HD:(hh + 1) * HD].T
        in_maps.append({
            'xT': xT, 'wqkvT': wqkvT,
            'pe_dq': pe_dq, 'pe_cq': pe_cq, 'pe_dk': pe_dk, 'pe_ck': pe_ck,
            'pw_rounds': np.ascontiguousarray(pw).astype(np.float16),
            'projb': projb,
        })
    return in_maps


_PROGRAM = None


def build_program():
    global _PROGRAM
    if _PROGRAM is not None:
        return _PROGRAM
    nc = bacc.Bacc("TRN2", target_bir_lowering=False, debug=False, num_devices=NC)
    ins = {
        'xT': nc.dram_tensor("xT", [D, L], F16, kind="ExternalInput").ap(),
        'wqkvT': nc.dram_tensor("wqkvT", [D, 576], F16, kind="ExternalInput").ap(),
        'pe_dq': nc.dram_tensor("pe_dq", [128, NLT * HD], F16, kind="ExternalInput").ap(),
        'pe_cq': nc.dram_tensor("pe_cq", [128, NLT * HD], F16, kind="ExternalInput").ap(),
        'pe_dk': nc.dram_tensor("pe_dk", [128, NLT * HD], F16, kind="ExternalInput").ap(),
        'pe_ck': nc.dram_tensor("pe_ck", [128, NLT * HD], F16, kind="ExternalInput").ap(),
        'pw_rounds': nc.dram_tensor("pw_rounds", [HL, 4, 128, D], F16, kind="ExternalInput").ap(),
        'projb': nc.dram_tensor("projb", [128, D], F32, kind="ExternalInput").ap(),
    }
    outs = {'out': nc.dram_tensor("out", [QTR, D], F32, kind="ExternalOutput").ap()}
    with tile.TileContext(nc) as tc:
        with ExitStack() as ctx:
            kernel_body(ctx, tc, outs, ins)
    nc.compile()
    _PROGRAM = nc
    return nc


def kernel(**inputs) -> np.ndarray:
    nc = build_program()
    in_maps = host_prep(**inputs)
    res = run_bass_kernel_spmd(nc, in_maps, core_ids=list(range(NC)),
                               trace=bool(int(os.environ.get("KERNEL_TRACE", "0"))))
    out = np.empty((B, L, D), np.float32)
    for core in range(NC):
        b, jq = core // 4, core % 4
        out[b, QTR * jq:QTR * (jq + 1), :] = res.results[core]['out']
    kernel.last_results = res
    return out
